# revision 15
# baseline (speedup 1.0000x reference)
"""GAT message-passing kernel for Trainium2, 8 NeuronCores.

Strategy (graph/data parallel, per sharding hint):
  - Nodes are partitioned contiguously across 8 cores (6250 nodes each).
  - Edges (plus one self-loop per node) are assigned to the core owning
    their destination node, sorted by destination, grouped into
    128-destination "windows", padded to a uniform number of 128-edge
    tiles per window so all cores run an identical (SPMD) program.
  - Per layer: every core computes x_proj|a_src|a_dst for its node shard
    (one matmul against an extended weight matrix), the x_proj|a_src
    table is AllGathered, then each core gathers source rows per edge
    with indirect DMA and accumulates softmax-weighted messages per
    destination window with one-hot matmuls in PSUM.  FFN + LayerNorms
    are done on node shards held resident in SBUF.
  - Layer-invariant edge quantities (a_edge = <W_edge @ edge_attr,
    att_edge>, incl. PyG 'mean' self-loop fill) are folded on the host
    into a per-edge 8-vector; attention softmax is computed without the
    per-destination max subtraction (mathematically identity; values
    are O(1) so fp32 exp is safe).
"""

import math
import os

import numpy as np

import concourse.bass as bass
import concourse.mybir as mybir
import concourse.tile as tile
from concourse.bass_utils import run_bass_kernel_spmd
from concourse.masks import make_identity

# problem dims (hardcoded per contract)
N, E, D, H, ED, L = 50000, 800000, 256, 8, 16, 6
C = D // H  # 32
DFF = 4 * D  # 1024
NEG_SLOPE = 0.2
EPS = 1e-5
NCORES = 8
P = 128

F32 = mybir.dt.float32
I32 = mybir.dt.int32
AX = mybir.AluOpType
AF = mybir.ActivationFunctionType

NEG_BIG = -1.0e30  # padded-edge a_edge => exp -> 0


# ---------------------------------------------------------------------------
# host-side preprocessing
# ---------------------------------------------------------------------------

def _prep(inputs, n_nodes, n_edges, n_layers, n_cores):
    """Builds per-core input maps + the compile-time config."""
    x = np.asarray(inputs["x"], np.float32)
    edge_index = np.asarray(inputs["edge_index"])
    edge_attr = np.asarray(inputs["edge_attr"], np.float32)
    W = np.asarray(inputs["W"], np.float32)
    att_src = np.asarray(inputs["att_src"], np.float32)
    att_dst = np.asarray(inputs["att_dst"], np.float32)
    att_edge = np.asarray(inputs["att_edge"], np.float32)
    W_edge = np.asarray(inputs["W_edge"], np.float32)
    gat_bias = np.asarray(inputs["bias"], np.float32)

    d = W.shape[0]
    h = att_src.shape[0]
    c = att_src.shape[1]

    nsh = n_nodes // n_cores
    assert nsh * n_cores == n_nodes
    nw = math.ceil(nsh / P)
    lw = nsh - (nw - 1) * P  # rows in last window

    # fold attention vectors into the projection:  m @ W_ext ->
    # [x_proj | a_src | a_dst]
    S_src = np.zeros((d, h), np.float32)
    S_dst = np.zeros((d, h), np.float32)
    for hh in range(h):
        S_src[hh * c:(hh + 1) * c, hh] = att_src[hh]
        S_dst[hh * c:(hh + 1) * c, hh] = att_dst[hh]
    W_ext = np.concatenate([W, W @ S_src, W @ S_dst], axis=1)  # [d, d+2h]

    # per-edge a_edge = sum_c (ea @ W_edge)[h,c] * att_edge[h,c] = ea @ V
    V = np.zeros((W_edge.shape[0], h), np.float32)
    for hh in range(h):
        V[:, hh] = W_edge[:, hh * c:(hh + 1) * c] @ att_edge[hh]
    src0 = edge_index[0].astype(np.int64)
    dst0 = edge_index[1].astype(np.int64)

    # self-loop edge_attr = mean of incoming real-edge attrs (PyG default)
    order = np.argsort(dst0, kind="stable")
    dst_s = dst0[order]
    src_s = src0[order]
    ea_sum = np.zeros((n_nodes, W_edge.shape[0]), np.float32)
    if n_edges > 0:
        starts = np.flatnonzero(np.r_[True, dst_s[1:] != dst_s[:-1]])
        sums = np.add.reduceat(edge_attr[order], starts, axis=0)
        ea_sum[dst_s[starts]] = sums
    deg = np.bincount(dst0, minlength=n_nodes).astype(np.float32)
    ea_mean = ea_sum / np.maximum(deg, 1.0)[:, None]

    a_edge_real = (edge_attr @ V).astype(np.float32)[order]  # dst-sorted
    a_edge_self = (ea_mean @ V).astype(np.float32)

    # build per-core padded edge structure
    # all (core, window) groups padded to the same T tiles
    counts = np.zeros((n_cores, nw), np.int64)
    widx_all = dst_s // P  # global window id = dst // P
    core_all = dst_s // nsh
    # count per (core, local window): dst within core: local w = (dst % nsh)//P
    lw_all = (dst_s - core_all * nsh) // P
    np.add.at(counts, (core_all, lw_all), 1)
    # self loops: node i adds 1 edge to its own (core, window)
    node_ids = np.arange(n_nodes, dtype=np.int64)
    sc = node_ids // nsh
    slw = (node_ids - sc * nsh) // P
    np.add.at(counts, (sc, slw), 1)
    T = int(math.ceil(counts.max() / P))

    src_arr = np.zeros((n_cores, nw, P, T), np.int32)
    dl_arr = np.zeros((n_cores, nw, P, T), np.float32)
    aed_arr = np.full((n_cores, nw, P, T, h), NEG_BIG, np.float32)

    # per (core, window): concatenate real (dst-sorted) edges + self loops
    for cc in range(n_cores):
        base = cc * nsh
        lo = np.searchsorted(dst_s, base)
        hi = np.searchsorted(dst_s, base + nsh)
        cdst = dst_s[lo:hi] - base
        csrc = src_s[lo:hi]
        caed = a_edge_real[lo:hi]
        cw = cdst // P
        wstarts = np.searchsorted(cw, np.arange(nw))
        wends = np.searchsorted(cw, np.arange(nw) + 1)
        for w in range(nw):
            wsz = lw if w == nw - 1 else P
            s, e = wstarts[w], wends[w]
            self_nodes = base + w * P + np.arange(wsz)
            srcs = np.concatenate([csrc[s:e], self_nodes])
            dls = np.concatenate([cdst[s:e] - w * P,
                                  np.arange(wsz, dtype=np.int64)])
            aeds = np.concatenate([caed[s:e], a_edge_self[self_nodes]], axis=0)
            k = srcs.shape[0]
            assert k <= P * T
            # edge j -> (partition j % P, tile j // P)
            pj = np.arange(k) % P
            tj = np.arange(k) // P
            src_arr[cc, w, pj, tj] = srcs
            dl_arr[cc, w, pj, tj] = dls
            aed_arr[cc, w, pj, tj] = aeds

    # ffn / ln weights
    w1 = np.ascontiguousarray(np.asarray(inputs["ffn_w1"], np.float32))
    b1 = np.asarray(inputs["ffn_b1"], np.float32)
    w2 = np.ascontiguousarray(np.asarray(inputs["ffn_w2"], np.float32))
    b2 = np.ascontiguousarray(np.asarray(inputs["ffn_b2"], np.float32))
    ln1g = np.ascontiguousarray(np.asarray(inputs["ln1_g"], np.float32))
    ln1b = np.ascontiguousarray(np.asarray(inputs["ln1_b"], np.float32))
    ln2g = np.ascontiguousarray(np.asarray(inputs["ln2_g"], np.float32))
    ln2b = np.ascontiguousarray(np.asarray(inputs["ln2_b"], np.float32))
    # b1 as per-partition columns for the transposed h1 layout:
    # b1c[l, p, mi] = b1[l, mi*P + p]
    nmi = DFF // P if d == D else w1.shape[2] // P
    b1c = np.ascontiguousarray(
        b1.reshape(n_layers, nmi, P).transpose(0, 2, 1))

    iota_row = np.broadcast_to(
        np.arange(P, dtype=np.float32)[None, :], (P, P)).copy()

    in_maps = []
    for cc in range(n_cores):
        in_maps.append({
            "x0": np.ascontiguousarray(x[cc * nsh:(cc + 1) * nsh]),
            "w_ext": np.ascontiguousarray(W_ext),
            "gat_bias": gat_bias.copy(),
            "w1_d": w1, "b1c_d": b1c, "w2_d": w2, "b2_d": b2,
            "ln1g_d": ln1g, "ln1b_d": ln1b,
            "ln2g_d": ln2g, "ln2b_d": ln2b,
            "iota_d": iota_row,
            "src_d": np.ascontiguousarray(src_arr[cc]),
            "dl_d": np.ascontiguousarray(dl_arr[cc]),
            "aed_d": np.ascontiguousarray(aed_arr[cc]),
        })
    cfg = dict(n_nodes=n_nodes, nsh=nsh, nw=nw, lw=lw, T=T,
               n_layers=n_layers, n_cores=n_cores, d=d, h=h,
               dff=w1.shape[2])
    return in_maps, cfg


# ---------------------------------------------------------------------------
# device kernel
# ---------------------------------------------------------------------------

def _legalize_single_wait(nc):
    """This walrus build allows at most one sync wait per instruction.

    Split extra waits onto standalone EventSemaphore instructions right
    before the owner (same engine => identical semantics).
    """
    def fix(blocks):
        n = 0
        for blk in blocks:
            newl = []
            for inst in list(blk.instructions):
                si = getattr(inst, "sync_info", None)
                ow = list(si.on_wait) if (si is not None and si.on_wait) else []
                if len(ow) > 1:
                    for j, wt in enumerate(ow[:-1]):
                        newl.append(mybir.InstEventSemaphore(
                            name=f"{inst.name}wf{j}",
                            sync_info=mybir.SyncInfo(on_wait=[wt],
                                                     on_update=[]),
                            engine=inst.engine,
                        ))
                    inst.sync_info = mybir.SyncInfo(
                        on_wait=[ow[-1]], on_update=list(si.on_update))
                    n += 1
                newl.append(inst)
            blk.instructions = newl
            subs = list(blk.blocks) if getattr(blk, "blocks", None) else []
            if subs:
                n += fix(subs)
        return n

    for f in nc.m.functions:
        fix(list(f.blocks))


def build_nc(cfg, legalize=True):
    n_nodes = cfg["n_nodes"]
    nsh = cfg["nsh"]
    NW = cfg["nw"]
    LW = cfg["lw"]
    T = cfg["T"]
    LAYERS = cfg["n_layers"]
    n_cores = cfg["n_cores"]
    d = cfg["d"]
    h = cfg["h"]
    dff = cfg["dff"]
    KT = d // P          # K-tiles for D-contraction (2)
    NMI = dff // P       # M-tiles for dff (8)
    TBL = d + h          # 264: x_proj | a_src
    EXT = d + 2 * h      # 272: x_proj | a_src | a_dst

    nc = bass.Bass("TRN2", target_bir_lowering=False, debug=False,
                   num_devices=n_cores)

    x0 = nc.dram_tensor("x0", [nsh, d], F32, kind="ExternalInput")
    w_ext = nc.dram_tensor("w_ext", [d, EXT], F32, kind="ExternalInput")
    gat_bias = nc.dram_tensor("gat_bias", [d], F32, kind="ExternalInput")
    w1_d = nc.dram_tensor("w1_d", [LAYERS, d, dff], F32, kind="ExternalInput")
    b1c_d = nc.dram_tensor("b1c_d", [LAYERS, P, NMI], F32, kind="ExternalInput")
    w2_d = nc.dram_tensor("w2_d", [LAYERS, dff, d], F32, kind="ExternalInput")
    b2_d = nc.dram_tensor("b2_d", [LAYERS, d], F32, kind="ExternalInput")
    ln1g_d = nc.dram_tensor("ln1g_d", [LAYERS, d], F32, kind="ExternalInput")
    ln1b_d = nc.dram_tensor("ln1b_d", [LAYERS, d], F32, kind="ExternalInput")
    ln2g_d = nc.dram_tensor("ln2g_d", [LAYERS, d], F32, kind="ExternalInput")
    ln2b_d = nc.dram_tensor("ln2b_d", [LAYERS, d], F32, kind="ExternalInput")
    iota_d = nc.dram_tensor("iota_d", [P, P], F32, kind="ExternalInput")
    src_d = nc.dram_tensor("src_d", [NW, P, T], I32, kind="ExternalInput")
    dl_d = nc.dram_tensor("dl_d", [NW, P, T], F32, kind="ExternalInput")
    aed_d = nc.dram_tensor("aed_d", [NW, P, T, h], F32, kind="ExternalInput")
    out_d = nc.dram_tensor("out_d", [nsh, d], F32, kind="ExternalOutput")

    def wsz(w):
        return LW if w == NW - 1 else P

    with tile.TileContext(nc) as tc:
        with (
            tc.tile_pool(name="const", bufs=1) as const,
            tc.tile_pool(name="wpool", bufs=1) as wpool,
            tc.tile_pool(name="big", bufs=1) as big,
            tc.tile_pool(name="work", bufs=2) as work,
            tc.tile_pool(name="small", bufs=2) as small,
            tc.tile_pool(name="stats", bufs=1) as stats,
            tc.tile_pool(name="psum", bufs=1, space="PSUM") as psum,
            tc.tile_pool(name="dram", bufs=1, space="DRAM") as dram,
        ):
            # ---------- constants ----------
            ident = const.tile([P, P], F32)
            make_identity(nc, ident[:, :])
            iota_sb = const.tile([P, P], F32)
            nc.sync.dma_start(out=iota_sb[:, :], in_=iota_d[:, :])
            wext_sb = const.tile([P, KT, EXT], F32)
            nc.sync.dma_start(
                out=wext_sb[:, :, :],
                in_=w_ext[:, :].rearrange("(kk p) c -> p kk c", p=P))
            bias_b = const.tile([P, d], F32)
            nc.sync.dma_start(out=bias_b[:, :],
                              in_=gat_bias[None, :].to_broadcast((P, d)))
            eps_t = const.tile([P, 1], F32)
            nc.vector.memset(eps_t[:, :], EPS)

            # ---------- persistent node state ----------
            m_sb = big.tile([P, NW, d], F32)
            if LW < P:
                nc.vector.memset(m_sb[:, NW - 1, :], 0.0)
            for w in range(NW):
                nc.sync.dma_start(out=m_sb[:wsz(w), w, :],
                                  in_=x0[w * P:w * P + wsz(w), :])
            g_all = big.tile([P, NW, TBL], F32)
            adst_sb = big.tile([P, NW, h], F32)

            for layer in range(LAYERS):
                # ---------- per-layer weights ----------
                w1_sb = wpool.tile([P, KT, dff], F32, tag="w1")
                nc.sync.dma_start(
                    out=w1_sb[:, :, :],
                    in_=w1_d[layer, :, :].rearrange("(kk p) f -> p kk f", p=P))
                w2_sb = wpool.tile([P, NMI, d], F32, tag="w2")
                nc.sync.dma_start(
                    out=w2_sb[:, :, :],
                    in_=w2_d[layer, :, :].rearrange("(kk p) f -> p kk f", p=P))
                b1c_sb = wpool.tile([P, NMI], F32, tag="b1c")
                nc.sync.dma_start(out=b1c_sb[:, :], in_=b1c_d[layer, :, :])
                b2_b = wpool.tile([P, d], F32, tag="b2")
                nc.sync.dma_start(
                    out=b2_b[:, :],
                    in_=b2_d[layer:layer + 1, :].to_broadcast((P, d)))
                ln1g_b = wpool.tile([P, d], F32, tag="ln1g")
                nc.sync.dma_start(
                    out=ln1g_b[:, :],
                    in_=ln1g_d[layer:layer + 1, :].to_broadcast((P, d)))
                ln1b_b = wpool.tile([P, d], F32, tag="ln1b")
                nc.sync.dma_start(
                    out=ln1b_b[:, :],
                    in_=ln1b_d[layer:layer + 1, :].to_broadcast((P, d)))
                ln2g_b = wpool.tile([P, d], F32, tag="ln2g")
                nc.sync.dma_start(
                    out=ln2g_b[:, :],
                    in_=ln2g_d[layer:layer + 1, :].to_broadcast((P, d)))
                ln2b_b = wpool.tile([P, d], F32, tag="ln2b")
                nc.sync.dma_start(
                    out=ln2b_b[:, :],
                    in_=ln2b_d[layer:layer + 1, :].to_broadcast((P, d)))

                shard_t = dram.tile([nsh, TBL], F32, tag="shard")
                table_t = dram.tile([n_nodes, TBL], F32, tag="table",
                                    addr_space="Shared")

                # ---------- phase A: x_proj | a_src | a_dst ----------
                for w in range(NW):
                    mT_sb = work.tile([P, KT, P], F32, tag="mT")
                    for kk in range(KT):
                        tp = psum.tile([P, P], F32, tag="t1", bufs=2)
                        nc.tensor.transpose(
                            tp[:, :], m_sb[:, w, kk * P:(kk + 1) * P],
                            ident[:, :])
                        nc.scalar.activation(mT_sb[:, kk, :], tp[:, :],
                                             AF.Copy)
                    pj = psum.tile([P, EXT], F32, tag="t2", bufs=2)
                    for kk in range(KT):
                        nc.tensor.matmul(pj[:, :], lhsT=mT_sb[:, kk, :],
                                         rhs=wext_sb[:, kk, :],
                                         start=(kk == 0), stop=(kk == KT - 1))
                    prj = work.tile([P, EXT], F32, tag="prj")
                    nc.scalar.activation(prj[:, :], pj[:, :], AF.Copy)
                    nc.sync.dma_start(out=shard_t[w * P:w * P + wsz(w), :],
                                      in_=prj[:wsz(w), :TBL])
                    nc.vector.tensor_copy(adst_sb[:, w, :], prj[:, TBL:EXT])

                # ---------- all-gather the projection table ----------
                if os.environ.get("GAT_ABL") != "noag":
                    nc.gpsimd.collective_compute(
                        "AllGather",
                        AX.bypass,
                        replica_groups=[list(range(n_cores))],
                        ins=[shard_t.opt()],
                        outs=[table_t.opt()],
                    )

                # ---------- phase B: edge aggregation ----------
                for w in range(NW):
                    idx_w = small.tile([P, T], I32, tag="idx")
                    nc.sync.dma_start(out=idx_w[:, :], in_=src_d[w, :, :])
                    dl_w = small.tile([P, T], F32, tag="dl")
                    nc.sync.dma_start(out=dl_w[:, :], in_=dl_d[w, :, :])
                    aed_w = work.tile([P, T, h], F32, tag="aed")
                    nc.sync.dma_start(out=aed_w[:, :, :], in_=aed_d[w, :, :, :])
                    gat_w = work.tile([P, T, TBL], F32, tag="gat")
                    if os.environ.get("GAT_ABL") != "nogather":
                        for t in range(T):
                            nc.gpsimd.indirect_dma_start(
                                out=gat_w[:, t, :],
                                out_offset=None,
                                in_=table_t[:, :],
                                in_offset=bass.IndirectOffsetOnAxis(
                                    ap=idx_w[:, t:t + 1], axis=0),
                            )
                    # one-hot [edge, windownode]
                    oh = work.tile([P, T, P], F32, tag="oh")
                    nc.vector.tensor_tensor(
                        out=oh[:, :, :],
                        in0=dl_w[:, :, None].to_broadcast((P, T, P)),
                        in1=iota_sb[:, None, :].to_broadcast((P, T, P)),
                        op=AX.is_equal)
                    # a_dst gathered per edge: adx = oh.T-expand @ a_dst
                    adx = psum.tile([P, T, h], F32, tag="t3", bufs=2)
                    for t in range(T):
                        tp2 = psum.tile([P, P], F32, tag="t1", bufs=2)
                        nc.tensor.transpose(tp2[:, :], oh[:, t, :],
                                            ident[:, :])
                        ohT = work.tile([P, P], F32, tag="ohT", bufs=3)
                        nc.scalar.activation(ohT[:, :], tp2[:, :], AF.Copy)
                        nc.tensor.matmul(adx[:, t, :], lhsT=ohT[:, :],
                                         rhs=adst_sb[:, w, :],
                                         start=True, stop=True)
                    # alpha -> leaky relu -> exp (into a_src slot of gat_w)
                    s1 = work.tile([P, T, h], F32, tag="s1")
                    nc.vector.tensor_tensor(out=s1[:, :, :],
                                            in0=gat_w[:, :, d:TBL],
                                            in1=aed_w[:, :, :], op=AX.add)
                    nc.vector.tensor_tensor(out=s1[:, :, :], in0=s1[:, :, :],
                                            in1=adx[:, :, :], op=AX.add)
                    # leaky relu = max(s, slope*s)
                    s2 = work.tile([P, T, h], F32, tag="s2")
                    nc.vector.tensor_scalar(out=s2[:, :, :], in0=s1[:, :, :],
                                            scalar1=NEG_SLOPE, scalar2=None,
                                            op0=AX.mult)
                    nc.vector.tensor_tensor(out=s2[:, :, :], in0=s1[:, :, :],
                                            in1=s2[:, :, :], op=AX.max)
                    nc.scalar.activation(gat_w[:, :, d:TBL], s2[:, :, :],
                                         AF.Exp)
                    # messages: x_proj *= ex (broadcast over channels)
                    nc.vector.tensor_tensor(
                        out=gat_w[:, :, 0:d].rearrange(
                            "p t (hh c) -> p t hh c", c=C),
                        in0=gat_w[:, :, 0:d].rearrange(
                            "p t (hh c) -> p t hh c", c=C),
                        in1=gat_w[:, :, d:TBL][:, :, :, None].to_broadcast(
                            (P, T, h, C)),
                        op=AX.mult)
                    # accumulate  [window, x_sum | ex_sum]
                    acc = psum.tile([P, TBL], F32, tag="t2", bufs=2)
                    for t in range(T):
                        nc.tensor.matmul(acc[:, :], lhsT=oh[:, t, :],
                                         rhs=gat_w[:, t, :],
                                         start=(t == 0), stop=(t == T - 1))
                    nc.scalar.activation(g_all[:, w, :], acc[:, :], AF.Copy)

                # ---------- batched epilogue: softmax div + bias + resid ----
                den = g_all[:, :, d:TBL]
                nc.vector.tensor_scalar(out=den, in0=den, scalar1=1e-30,
                                        scalar2=None, op0=AX.max)
                rec = stats.tile([P, NW, h], F32, tag="rec")
                nc.vector.reciprocal(rec[:, :, :], den)
                nc.vector.tensor_tensor(
                    out=g_all[:, :, 0:d].rearrange(
                        "p w (hh c) -> p w hh c", c=C),
                    in0=g_all[:, :, 0:d].rearrange(
                        "p w (hh c) -> p w hh c", c=C),
                    in1=rec[:, :, :, None].to_broadcast((P, NW, h, C)),
                    op=AX.mult)
                nc.vector.tensor_tensor(
                    out=g_all[:, :, 0:d], in0=g_all[:, :, 0:d],
                    in1=bias_b[:, None, :].to_broadcast((P, NW, d)),
                    op=AX.add)
                nc.vector.tensor_tensor(out=m_sb[:, :, :],
                                        in0=m_sb[:, :, :],
                                        in1=g_all[:, :, 0:d], op=AX.add)

                # ---------- LN1 (batched, E[x^2]-mu^2) ----------
                _layernorm_batched(nc, stats, m_sb, g_all, ln1g_b, ln1b_b,
                                   eps_t, NW, d)

                # ---------- phase C: FFN ----------
                for w in range(NW):
                    mT_sb = work.tile([P, KT, P], F32, tag="mT")
                    for kk in range(KT):
                        tp = psum.tile([P, P], F32, tag="t1", bufs=2)
                        nc.tensor.transpose(
                            tp[:, :], m_sb[:, w, kk * P:(kk + 1) * P],
                            ident[:, :])
                        nc.scalar.activation(mT_sb[:, kk, :], tp[:, :],
                                             AF.Copy)
                    h1T = work.tile([P, NMI, P], F32, tag="h1T")
                    for mi in range(NMI):
                        tag = "t2" if mi % 2 == 0 else "t3"
                        hp = psum.tile([P, P], F32, tag=tag, bufs=2)
                        for kk in range(KT):
                            nc.tensor.matmul(
                                hp[:, :],
                                lhsT=w1_sb[:, kk, mi * P:(mi + 1) * P],
                                rhs=mT_sb[:, kk, :],
                                start=(kk == 0), stop=(kk == KT - 1))
                        nc.scalar.activation(h1T[:, mi, :], hp[:, :], AF.Relu,
                                             bias=b1c_sb[:, mi:mi + 1])
                    h2p = psum.tile([P, d], F32, tag="t1", bufs=2)
                    for mi in range(NMI):
                        nc.tensor.matmul(h2p[:, :], lhsT=h1T[:, mi, :],
                                         rhs=w2_sb[:, mi, :],
                                         start=(mi == 0), stop=(mi == NMI - 1))
                    nc.scalar.activation(g_all[:, w, 0:d], h2p[:, :], AF.Copy)

                # h + b2 + m; LN2
                nc.vector.tensor_tensor(
                    out=g_all[:, :, 0:d], in0=g_all[:, :, 0:d],
                    in1=b2_b[:, None, :].to_broadcast((P, NW, d)), op=AX.add)
                nc.vector.tensor_tensor(out=m_sb[:, :, :], in0=m_sb[:, :, :],
                                        in1=g_all[:, :, 0:d], op=AX.add)
                _layernorm_batched(nc, stats, m_sb, g_all, ln2g_b, ln2b_b,
                                   eps_t, NW, d)

            # ---------- output ----------
            for w in range(NW):
                nc.sync.dma_start(out=out_d[w * P:w * P + wsz(w), :],
                                  in_=m_sb[:wsz(w), w, :])
    if legalize:
        _legalize_single_wait(nc)
    return nc


def _layernorm_batched(nc, stats, m_sb, g_all, g_b, b_b, eps_t, NW, d):
    """In-place LayerNorm over the feature axis for all windows at once.

    Uses var = E[x^2] - mu^2; g_all[:, :, 0:d] is free scratch here.
    """
    ssum = stats.tile([P, NW], F32, tag="ssum")
    nc.vector.tensor_reduce(out=ssum[:, :], in_=m_sb[:, :, :],
                            axis=mybir.AxisListType.X, op=AX.add)
    sq = g_all[:, :, 0:d]
    nc.vector.tensor_tensor(out=sq, in0=m_sb[:, :, :], in1=m_sb[:, :, :],
                            op=AX.mult)
    ssq = stats.tile([P, NW], F32, tag="ssq")
    nc.vector.tensor_reduce(out=ssq[:, :], in_=sq,
                            axis=mybir.AxisListType.X, op=AX.add)
    mu = stats.tile([P, NW], F32, tag="mu")
    nc.vector.tensor_scalar(out=mu[:, :], in0=ssum[:, :], scalar1=1.0 / d,
                            scalar2=None, op0=AX.mult)
    var = stats.tile([P, NW], F32, tag="var")
    nc.vector.tensor_scalar(out=var[:, :], in0=ssq[:, :], scalar1=1.0 / d,
                            scalar2=None, op0=AX.mult)
    mu2 = stats.tile([P, NW], F32, tag="mu2")
    nc.vector.tensor_tensor(out=mu2[:, :], in0=mu[:, :], in1=mu[:, :],
                            op=AX.mult)
    nc.vector.tensor_tensor(out=var[:, :], in0=var[:, :], in1=mu2[:, :],
                            op=AX.subtract)
    # rstd = 1/sqrt(var+eps)
    nc.scalar.activation(var[:, :], var[:, :], AF.Sqrt, bias=eps_t[:, :1])
    rstd = stats.tile([P, NW], F32, tag="rstd")
    nc.vector.reciprocal(rstd[:, :], var[:, :])
    # apply
    nc.vector.tensor_tensor(out=m_sb[:, :, :], in0=m_sb[:, :, :],
                            in1=mu[:, :, None].to_broadcast((P, NW, d)),
                            op=AX.subtract)
    nc.vector.tensor_tensor(out=m_sb[:, :, :], in0=m_sb[:, :, :],
                            in1=rstd[:, :, None].to_broadcast((P, NW, d)),
                            op=AX.mult)
    nc.vector.tensor_tensor(out=m_sb[:, :, :], in0=m_sb[:, :, :],
                            in1=g_b[:, None, :].to_broadcast((P, NW, d)),
                            op=AX.mult)
    nc.vector.tensor_tensor(out=m_sb[:, :, :], in0=m_sb[:, :, :],
                            in1=b_b[:, None, :].to_broadcast((P, NW, d)),
                            op=AX.add)


# ---------------------------------------------------------------------------
# entry point
# ---------------------------------------------------------------------------

_LAST_RESULTS = {}


def kernel(**inputs):
    n_nodes = inputs["x"].shape[0]
    n_edges = inputs["edge_index"].shape[1]
    n_layers = inputs["ffn_w1"].shape[0]
    in_maps, cfg = _prep(inputs, n_nodes, n_edges, n_layers, NCORES)
    nc = build_nc(cfg)
    res = run_bass_kernel_spmd(
        nc, in_maps, list(range(NCORES)),
        trace=bool(int(os.environ.get("GAT_TRACE", "0"))),
    )
    _LAST_RESULTS["res"] = res
    out = np.concatenate([res.results[cc]["out_d"] for cc in range(NCORES)],
                         axis=0)
    return out
